# revision 2
# baseline (speedup 1.0000x reference)
"""Trainium2 Bass kernel v2 for nn_DenseRMoK — latency-optimized redesign.

Key differences from the v1 (styles A-H) kernel:
- bf16 for x, weights, and all chunked intermediates (xn/x2/psi): halves DMA
  traffic and unlocks the DVE 2x mode; PSUM accumulation stays f32.
- No ACT table swaps: sqrt(var+eps) is computed as exp(+/-0.5*ln(var+eps)),
  so the whole kernel lives in the natural_log_exp activation-function set
  (exp/ln/copy/square/identity), eliminating ~5us of LoadActFuncSet stalls.
- Stats no longer depend on the weight DMA: the ones/1/L matmul columns are
  memset on-chip.
- No transpose dance in the tail: the per-row score/denorm scalars are
  broadcast across partitions with K=1 matmuls and the expert mixture is a
  handful of elementwise ops in feature-major layout; expert biases are
  folded into the mix via scalar_tensor_tensor's per-partition scalar port.
- Specializes rev_w==1 / rev_b==0 (checked at runtime; exact numpy fallback
  otherwise, as for the wave scale/trans identity).
"""

import math
import sys

import numpy as np

if "/opt/trn_rl_repo" not in sys.path:
    sys.path.insert(0, "/opt/trn_rl_repo")

B, L, N, P, E = 32, 512, 64, 96, 4
EPS = 1e-5
BN_EPS = 1e-5
MH = 2.0 / (math.sqrt(3.0) * math.pi**0.25)

NCORES = 8
BPC = B // NCORES
R = BPC * N          # 256 rows per core
PD = 128
NCH = L // PD        # 4 l-chunks
WCOLS = NCH * P      # 384 cols per chunked [L,P] weight
WTOT = 6 * WCOLS + NCH * E  # 2320: c10|c20|c11|c21|ww0|ww1|wg

_NC_CACHE = {}

STYLES = {
    # engine assignment: front sq, mid x2, psib
    "A": dict(sq="vector", x2="gpsimd", psib="vector"),
    "B": dict(sq="vector", x2="act", psib="vector"),
    "C": dict(sq="gpsimd", x2="vector", psib="gpsimd"),
    "D": dict(sq="vector", x2="vector", psib="gpsimd"),
}


def _build_nc(debug=False, loop_n=1, style="D"):
    from contextlib import nullcontext

    import concourse.tile as tile
    from concourse import bacc, mybir
    from concourse._compat import get_trn_type
    from concourse.hw_specs import get_activation_tables

    st = STYLES[style] if isinstance(style, str) else style

    f32 = mybir.dt.float32
    f32r = mybir.dt.float32r
    bf16 = mybir.dt.bfloat16
    AF = mybir.ActivationFunctionType
    OP = mybir.AluOpType

    nc = bacc.Bacc(get_trn_type() or "TRN2", target_bir_lowering=False, debug=debug)

    # activation-function set with exp+ln co-resident (also copy/square/
    # identity): one explicit load, no per-iteration table thrash
    tables = list(get_activation_tables(nc.m.arch).items())
    actset = next(
        i for i, (_, fs) in enumerate(tables) if AF.Exp in fs and AF.Ln in fs
    )

    x_d = nc.dram_tensor("x", [PD, NCH * R], bf16, kind="ExternalInput")
    w_d = nc.dram_tensor("w", [PD, WTOT], bf16, kind="ExternalInput")
    aux_d = nc.dram_tensor("aux", [1, E * P + E], f32r, kind="ExternalInput")
    sel_d = nc.dram_tensor("sel", [E, E * P], f32r, kind="ExternalInput")
    out_d = nc.dram_tensor("out", [P, R], f32, kind="ExternalOutput")

    B0 = -math.log1p(EPS)  # folds the 1/(rev_w+eps)=1/(1+eps) denorm scale

    def eng(name):
        return {"vector": nc.vector, "gpsimd": nc.gpsimd, "act": nc.scalar}[name]

    with tile.TileContext(nc) as tc:
        with (
            tc.tile_pool(name="const", bufs=1) as cp,
            tc.tile_pool(name="big", bufs=2) as bp,
            tc.tile_pool(name="sm", bufs=2) as sp,
            tc.tile_pool(name="psA", bufs=1, space="PSUM") as psA_p,
            tc.tile_pool(name="psB", bufs=1, space="PSUM") as psB_p,
            tc.tile_pool(name="psac", bufs=1, space="PSUM") as psac_p,
            tc.tile_pool(name="psacc", bufs=1, space="PSUM") as psacc_p,
            tc.tile_pool(name="psg", bufs=1, space="PSUM") as psg_p,
        ):
            # --- one-time constants + act table load (outside the timing loop) ---
            onesR = cp.tile([1, R], f32r, tag="onesR")      # all-ones lhsT/rhs row
            onesRf = cp.tile([1, R], f32, tag="onesRf")
            invL = cp.tile([PD, 1], bf16, tag="invL")       # stats lhsT col
            invLf = cp.tile([PD, 1], f32, tag="invLf")
            ones4 = cp.tile([E, 1], f32r, tag="ones4")      # score-sum lhsT
            ones4f = cp.tile([E, 1], f32, tag="ones4f")
            cbias = cp.tile([1, 2], f32, tag="cbias")
            zbias = cp.tile([PD, 1], f32, tag="zbias")
            nc.gpsimd.memset(onesRf, 1.0)
            nc.vector.tensor_copy(onesR, onesRf)
            nc.gpsimd.memset(invLf, 1.0 / L)
            nc.vector.tensor_copy(invL, invLf)
            nc.gpsimd.memset(ones4f, 1.0)
            nc.vector.tensor_copy(ones4, ones4f)
            nc.gpsimd.memset(cbias[:, 0:1], EPS)
            nc.gpsimd.memset(cbias[:, 1:2], B0)
            nc.vector.memset(zbias, 0.0)
            ones_r = onesR[:, 0:PD]
            sel = cp.tile([E, E * P], f32r, tag="sel")      # one-hot row selectors
            nc.scalar.dma_start(out=sel, in_=sel_d[:])
            ld = mybir.InstLoadActFuncSet(
                name=nc.get_next_instruction_name(), ins=[], outs=[]
            )
            ld.act_func_set_id = actset
            ld.engine = nc.scalar.engine
            nc.scalar.add_instruction(ld)

            with tc.For_i(0, loop_n, 1) if loop_n > 1 else nullcontext():
                # --- DMAs: x on sync ring; aux row + weights on scalar ring ---
                xs = bp.tile([PD, NCH * R], bf16, tag="xs")
                wsb = cp.tile([PD, WTOT], bf16, tag="wsb")
                aux = cp.tile([1, E * P + E], f32r, tag="aux")
                nc.sync.dma_start(out=xs, in_=x_d[:])
                nc.scalar.dma_start(out=aux, in_=aux_d[:])
                nc.scalar.dma_start(out=wsb, in_=w_d[:])

                c10 = wsb[:, 0 * WCOLS : 1 * WCOLS]
                c20 = wsb[:, 1 * WCOLS : 2 * WCOLS]
                c11 = wsb[:, 2 * WCOLS : 3 * WCOLS]
                c21 = wsb[:, 3 * WCOLS : 4 * WCOLS]
                ww0 = wsb[:, 4 * WCOLS : 5 * WCOLS]
                ww1 = wsb[:, 5 * WCOLS : 6 * WCOLS]
                wg = wsb[:, 6 * WCOLS : 6 * WCOLS + NCH * E]
                gbrow = aux[:, E * P : E * P + E]

                # --- RevIN stats: mean | E[x^2] via accumulating 1/L matmuls ---
                sq = bp.tile([PD, NCH * R], bf16, tag="sq")
                bankA = psA_p.tile([PD, 2 * R], f32, tag="bankA")
                bankB = psB_p.tile([PD, 2 * R], f32, tag="bankB")
                mean_ps = bankA[0:1, 0:R]
                ex2_ps = bankB[0:1, 0:R]
                for c in range(NCH):
                    cs = slice(c * R, (c + 1) * R)
                    nc.vector.tensor_mul(sq[:, cs], xs[:, cs], xs[:, cs])
                    nc.tensor.matmul(
                        mean_ps, invL, xs[:, cs], start=(c == 0), stop=(c == NCH - 1)
                    )
                    nc.tensor.matmul(
                        ex2_ps, invL, sq[:, cs], start=(c == 0), stop=(c == NCH - 1)
                    )

                # --- istd/stdev chain (exp/ln in place of sqrt) ---
                tsm = sp.tile([1, R], f32, tag="tsm")
                varsm = sp.tile([1, R], f32, tag="varsm")
                lvsm = sp.tile([1, R], f32, tag="lvsm")
                sdrsm = sp.tile([1, R], f32, tag="sdrsm")
                acr = sp.tile([1, 2 * R], f32r, tag="acr")    # [istd | -mean]
                dmrow = sp.tile([1, R], f32r, tag="dmrow")      # dar row
                pac = psac_p.tile([PD, 2 * R], f32, tag="pac")  # [A bcast | CM bcast]
                ACsb = bp.tile([PD, 2 * R], bf16, tag="ACsb")
                # CM = bcast(-mean) is ready long before istd: the centering
                # adds run during the ln/exp chain
                nc.vector.tensor_scalar_mul(acr[:, R:], mean_ps, -1.0)
                nc.tensor.matmul(pac[:, R:], ones_r, acr[:, R:], start=True, stop=True)
                nc.scalar.activation(ACsb[:, R:], pac[:, R:], AF.Copy)
                nc.scalar.activation(tsm, mean_ps, AF.Square, bias=zbias[0:1])
                nc.vector.scalar_tensor_tensor(
                    varsm, tsm, -1.0, ex2_ps, op0=OP.mult, op1=OP.add
                )
                nc.scalar.activation(lvsm, varsm, AF.Ln, bias=cbias[:, 0:1])
                nc.scalar.activation(acr[:, 0:R], lvsm, AF.Exp, scale=-0.5)
                nc.tensor.matmul(pac[:, 0:R], ones_r, acr[:, 0:R], start=True, stop=True)
                nc.scalar.activation(ACsb[:, 0:R], pac[:, 0:R], AF.Copy)
                nc.scalar.activation(sdrsm, lvsm, AF.Exp, bias=cbias[:, 1:2], scale=0.5)

                # --- main chunk loop ---
                xsc = bp.tile([PD, NCH * R], bf16, tag="xsc")
                xn = bp.tile([PD, NCH * R], bf16, tag="xn")
                x2 = bp.tile([PD, NCH * R], bf16, tag="x2")
                eT = bp.tile([PD, NCH * R], bf16, tag="eT")
                psib = bp.tile([PD, NCH * R], bf16, tag="psib")
                ptw = psacc_p.tile([P, 4 * 2 * R], f32, tag="ptw")  # 4 banks
                pt0 = ptw[:, 0 * 2 * R : 0 * 2 * R + R]
                pt1 = ptw[:, 1 * 2 * R : 1 * 2 * R + R]
                pw0 = ptw[:, 2 * 2 * R : 2 * 2 * R + R]
                pw1 = ptw[:, 3 * 2 * R : 3 * 2 * R + R]
                pgt = psg_p.tile([E, 2 * R], f32, tag="pgt")
                pg = pgt[:, 0:R]
                prs = pgt[0:1, R : R + R]
                # late, time-disjoint aliases into the stats/bcast banks
                pdadc = pac[0:P, 0:R]
                pG01 = bankA[0:P, :]
                pG23 = bankB[0:P, :]
                pGs = [pG01[:, 0:R], pG01[:, R:], pG23[:, 0:R], pG23[:, R:]]
                expg = sp.tile([E, R], f32r, tag="expg")
                rinvsm = sp.tile([1, R], f32, tag="rinvsm")
                G01sb = bp.tile([P, 2 * R], bf16, tag="G01sb")
                G23sb = bp.tile([P, 2 * R], bf16, tag="G23sb")
                DAsb = bp.tile([P, R], bf16, tag="DAsb")

                x2_e = eng(st["x2"])
                psib_e = eng(st["psib"])
                bmm = [aux[:, e * P : (e + 1) * P] for e in range(E)]
                # centering (xs - mean) for all chunks: overlaps the istd chain
                for c in range(NCH):
                    cs = slice(c * R, (c + 1) * R)
                    CM_ = pac[:, R:] if c == 0 else ACsb[:, R:]
                    xe = nc.vector if c == 0 else nc.gpsimd
                    xe.tensor_add(xsc[:, cs], xs[:, cs], CM_)
                # pass A: xn/x2/eT + gate & taylor matmuls (xn/x2-ready).
                # Wave matmuls are deferred to pass B so the PE's in-order
                # stream reaches the gate close + score broadcasts right
                # after xn3 instead of behind the psi backlog.
                for c in range(NCH):
                    cs = slice(c * R, (c + 1) * R)
                    cp_ = slice(c * P, (c + 1) * P)
                    A_ = pac[:, 0:R] if c == 0 else ACsb[:, 0:R]
                    last = c == NCH - 1
                    nc.vector.tensor_mul(xn[:, cs], xsc[:, cs], A_)
                    if c == 0:
                        nc.tensor.matmul(pg, gbrow, onesR, start=True, stop=False)
                    nc.tensor.matmul(
                        pg, wg[:, c * E : (c + 1) * E], xn[:, cs],
                        start=False, stop=last,
                    )
                    x2e = nc.gpsimd if c < NCH - 1 else nc.vector
                    x2e.tensor_mul(x2[:, cs], xn[:, cs], xn[:, cs])
                    nc.scalar.activation(eT[:, cs], x2[:, cs], AF.Exp,
                                         bias=zbias, scale=-0.5)
                    nc.tensor.matmul(pt0, c10[:, cp_], xn[:, cs],
                                     start=(c == 0), stop=False)
                    nc.tensor.matmul(pt1, c11[:, cp_], xn[:, cs],
                                     start=(c == 0), stop=False)
                    nc.tensor.matmul(pt0, c20[:, cp_], x2[:, cs],
                                     start=False, stop=False)
                    nc.tensor.matmul(pt1, c21[:, cp_], x2[:, cs],
                                     start=False, stop=(not last) and False)
                    if last:
                        # close taylor groups (bias rank-1 updates), then the
                        # score chain: everything the mix needs except waves
                        nc.tensor.matmul(pt0, bmm[0], onesR, start=False, stop=True)
                        nc.tensor.matmul(pt1, bmm[1], onesR, start=False, stop=True)
                        nc.scalar.activation(expg, pg, AF.Exp, bias=zbias[0:E])
                        nc.tensor.matmul(prs, ones4, expg, start=True, stop=True)
                        for e in range(E):
                            nc.tensor.matmul(pGs[e], sel[:, e * P : (e + 1) * P],
                                             expg, start=True, stop=True)
                        nc.vector.reciprocal(rinvsm, prs)
                        nc.vector.tensor_mul(dmrow, sdrsm, rinvsm)
                        nc.tensor.matmul(pdadc, ones_r[:, 0:P], dmrow,
                                         start=True, stop=True)
                        nc.scalar.activation(G01sb, pG01, AF.Copy)
                        nc.scalar.activation(G23sb, pG23, AF.Copy)
                        nc.scalar.activation(DAsb, pdadc, AF.Copy)
                # pass B: psi + wave matmuls (terminal chain)
                for c in range(NCH):
                    cs = slice(c * R, (c + 1) * R)
                    cp_ = slice(c * P, (c + 1) * P)
                    last = c == NCH - 1
                    nc.vector.scalar_tensor_tensor(
                        psib[:, cs], x2[:, cs], -1.0, eT[:, cs],
                        op0=OP.add, op1=OP.mult,
                    )
                    nc.tensor.matmul(pw0, ww0[:, cp_], psib[:, cs],
                                     start=(c == 0), stop=False)
                    nc.tensor.matmul(pw1, ww1[:, cp_], psib[:, cs],
                                     start=(c == 0), stop=False)
                    if last:
                        nc.tensor.matmul(pw0, bmm[2], onesR, start=False, stop=True)
                        nc.tensor.matmul(pw1, bmm[3], onesR, start=False, stop=True)

                # --- mixture + denorm, feature-major ---
                # taylor-side pre = s1*DA + mean on DVE+Pool (early, off the
                # terminal path); wave side m23 -> s2 -> s2*DA -> +pre on DVE
                m0 = bp.tile([P, R], f32, tag="m0")
                m1 = bp.tile([P, R], f32, tag="m1")
                m2 = bp.tile([P, R], f32, tag="m2")
                m3 = bp.tile([P, R], f32, tag="m3")
                s1 = bp.tile([P, R], f32, tag="s1")
                s2 = bp.tile([P, R], f32, tag="s2")
                pre = bp.tile([P, R], f32, tag="pre")
                outp = bp.tile([P, R], f32, tag="outp")
                nc.vector.tensor_mul(m0, pt0, G01sb[:, 0:R])
                nc.vector.tensor_mul(m1, pt1, G01sb[:, R:])
                nc.vector.tensor_add(s1, m0, m1)
                nc.gpsimd.tensor_mul(s1, s1, DAsb)
                nc.gpsimd.tensor_sub(pre, s1, ACsb[0:P, R:])
                nc.vector.tensor_mul(m2, pw0, G23sb[:, 0:R])
                nc.vector.tensor_mul(m3, pw1, G23sb[:, R:])
                nc.vector.tensor_add(s2, m2, m3)
                nc.vector.tensor_mul(s2, s2, pdadc)
                nc.vector.tensor_add(outp, s2, pre)
                nc.sync.dma_start(out=out_d[:], in_=outp)

    nc.compile()
    return nc


def _chunked(wT):
    """[L, M] -> [128, NCH*M], column block c holds rows l = c*128..(c+1)*128."""
    Lx, M = wT.shape
    return np.ascontiguousarray(
        wT.reshape(NCH, PD, M).transpose(1, 0, 2).reshape(PD, NCH * M)
    )


def _host_prep(inputs):
    import ml_dtypes

    f = np.float32
    bf = ml_dtypes.bfloat16
    g = {k: np.asarray(v, f) for k, v in inputs.items()}

    bn_scale = MH / math.sqrt(1.0 + BN_EPS)
    wparts = [
        _chunked(np.ascontiguousarray(g["t0_coeffs"][:, :, 1].T)),
        _chunked(np.ascontiguousarray(g["t0_coeffs"][:, :, 2].T)),
        _chunked(np.ascontiguousarray(g["t1_coeffs"][:, :, 1].T)),
        _chunked(np.ascontiguousarray(g["t1_coeffs"][:, :, 2].T)),
        _chunked(
            np.ascontiguousarray((g["w0_ww"] * g["w0_gamma"][:, None] * bn_scale).T)
        ),
        _chunked(
            np.ascontiguousarray((g["w1_ww"] * g["w1_gamma"][:, None] * bn_scale).T)
        ),
        _chunked(np.ascontiguousarray(g["gate_w"].T)),
    ]
    w_h = np.concatenate(wparts, axis=1).astype(bf)
    assert w_h.shape == (PD, WTOT)

    aux_h = np.zeros((1, E * P + E), f)
    aux_h[0, 0:P] = (
        g["t0_coeffs"][:, :, 0].sum(axis=1, dtype=np.float64) + g["t0_bias"][0]
    ).astype(f)
    aux_h[0, P : 2 * P] = (
        g["t1_coeffs"][:, :, 0].sum(axis=1, dtype=np.float64) + g["t1_bias"][0]
    ).astype(f)
    aux_h[0, 2 * P : 3 * P] = g["w0_beta"]
    aux_h[0, 3 * P : 4 * P] = g["w1_beta"]
    aux_h[0, 4 * P : 4 * P + E] = g["gate_b"] + np.float32(math.log1p(EPS))

    sel_h = np.zeros((E, E * P), f)
    for e in range(E):
        sel_h[e, e * P : (e + 1) * P] = 1.0
    common = {"w": w_h, "aux": aux_h, "sel": sel_h}

    x = g["x"]
    xcores = []
    for i in range(NCORES):
        xc = x[i * BPC : (i + 1) * BPC]  # [BPC, L, N]
        xcores.append(
            np.ascontiguousarray(
                xc.reshape(BPC, NCH, PD, N).transpose(2, 1, 0, 3).reshape(PD, NCH * R)
            ).astype(bf)
        )
    return common, xcores


def _fast_ok(inputs):
    try:
        return (
            np.all(np.asarray(inputs["w0_scale"]) == 1.0)
            and np.all(np.asarray(inputs["w1_scale"]) == 1.0)
            and np.all(np.asarray(inputs["w0_trans"]) == 0.0)
            and np.all(np.asarray(inputs["w1_trans"]) == 0.0)
            and np.all(np.asarray(inputs["rev_w"]) == 1.0)
            and np.all(np.asarray(inputs["rev_b"]) == 0.0)
        )
    except Exception:
        return False


def _numpy_ref(inputs):
    """Exact general fallback (host numpy), mirrors the reference module."""
    g = {k: np.asarray(v, np.float32) for k, v in inputs.items()}
    x = g["x"]
    mean = x.mean(axis=1, keepdims=True)
    stdev = np.sqrt(x.var(axis=1, keepdims=True) + np.float32(EPS))
    xn = (x - mean) / stdev * g["rev_w"] + g["rev_b"]
    xf = xn.transpose(0, 2, 1).reshape(B * N, L)
    logits = xf @ g["gate_w"].T + g["gate_b"]
    logits -= logits.max(axis=-1, keepdims=True)
    elg = np.exp(logits)
    score = elg / elg.sum(axis=-1, keepdims=True)

    def taylor(c, b):
        y = np.full((B * N, P), c[:, :, 0].sum(axis=1), np.float32)
        y += xf @ c[:, :, 1].T + (xf * xf) @ c[:, :, 2].T
        return y + b

    def wave(s, t, w, gam, bet):
        y = np.empty((B * N, P), np.float32)
        for i0 in range(0, B * N, 128):
            xs = (xf[i0 : i0 + 128, None, :] - t[None]) / s[None]
            x2 = xs * xs
            psi = np.float32(MH) * (x2 - 1.0) * np.exp(-0.5 * x2)
            y[i0 : i0 + 128] = np.einsum("bpl,pl->bp", psi, w)
        return (y / np.sqrt(np.float32(1.0 + BN_EPS))) * gam + bet

    eo = np.stack(
        [
            taylor(g["t0_coeffs"], g["t0_bias"][0]),
            taylor(g["t1_coeffs"], g["t1_bias"][0]),
            wave(g["w0_scale"], g["w0_trans"], g["w0_ww"], g["w0_gamma"], g["w0_beta"]),
            wave(g["w1_scale"], g["w1_trans"], g["w1_ww"], g["w1_gamma"], g["w1_beta"]),
        ],
        axis=-1,
    )
    pred = np.einsum("bpE,bE->bp", eo, score)
    pred = pred.reshape(B, N, P).transpose(0, 2, 1)
    out = ((pred - g["rev_b"]) / (g["rev_w"] + np.float32(EPS))) * stdev + mean
    return out.astype(np.float32)


def run(inputs, trace=False):
    """Run the Bass kernel on 8 cores. Returns (out [B,P,N], exec_time_ns|None)."""
    from concourse.bass_utils import run_bass_kernel_spmd

    if "nc" not in _NC_CACHE:
        _NC_CACHE["nc"] = _build_nc()
    nc = _NC_CACHE["nc"]
    common, xcores = _host_prep(inputs)
    in_maps = [dict(common, x=xcores[i]) for i in range(NCORES)]
    try:
        res = run_bass_kernel_spmd(nc, in_maps, list(range(NCORES)), trace=trace)
    except ModuleNotFoundError:
        res = run_bass_kernel_spmd(nc, in_maps, list(range(NCORES)), trace=False)
    out = np.empty((B, P, N), np.float32)
    for i in range(NCORES):
        o = np.asarray(res.results[i]["out"]).reshape(P, BPC, N)
        out[i * BPC : (i + 1) * BPC] = o.transpose(1, 0, 2)
    return out, res.exec_time_ns


def kernel(**inputs):
    if not _fast_ok(inputs):
        return _numpy_ref(inputs)
    out, _ = run(inputs)
    return out


# revision 4
# speedup vs baseline: 1.0194x; 1.0194x over previous
"""Trainium2 Bass kernel v2 for nn_DenseRMoK — latency-optimized redesign.

Key differences from the v1 (styles A-H) kernel:
- bf16 for x, weights, and all chunked intermediates (xn/x2/psi): halves DMA
  traffic and unlocks the DVE 2x mode; PSUM accumulation stays f32.
- No ACT table swaps: sqrt(var+eps) is computed as exp(+/-0.5*ln(var+eps)),
  so the whole kernel lives in the natural_log_exp activation-function set
  (exp/ln/copy/square/identity), eliminating ~5us of LoadActFuncSet stalls.
- Stats no longer depend on the weight DMA: the ones/1/L matmul columns are
  memset on-chip.
- No transpose dance in the tail: the per-row score/denorm scalars are
  broadcast across partitions with K=1 matmuls and the expert mixture is a
  handful of elementwise ops in feature-major layout; expert biases are
  folded into the mix via scalar_tensor_tensor's per-partition scalar port.
- Specializes rev_w==1 / rev_b==0 (checked at runtime; exact numpy fallback
  otherwise, as for the wave scale/trans identity).
"""

import math
import sys

import numpy as np

if "/opt/trn_rl_repo" not in sys.path:
    sys.path.insert(0, "/opt/trn_rl_repo")

B, L, N, P, E = 32, 512, 64, 96, 4
EPS = 1e-5
BN_EPS = 1e-5
MH = 2.0 / (math.sqrt(3.0) * math.pi**0.25)

NCORES = 8
BPC = B // NCORES
R = BPC * N          # 256 rows per core
PD = 128
NCH = L // PD        # 4 l-chunks
WCOLS = NCH * P      # 384 cols per chunked [L,P] weight
WTOT = 6 * WCOLS + NCH * E  # 2320: c10|c20|c11|c21|ww0|ww1|wg
WAUX = WTOT + E * P + E     # + bias rows (partition 0) + gate bias

_NC_CACHE = {}

STYLES = {
    # engine assignment: front sq, mid x2, psib
    "A": dict(sq="vector", x2="gpsimd", psib="vector"),
    "B": dict(sq="vector", x2="act", psib="vector"),
    "C": dict(sq="gpsimd", x2="vector", psib="gpsimd"),
    "D": dict(sq="vector", x2="vector", psib="gpsimd"),
}


def _build_nc(debug=False, loop_n=1, style="D", STAGGER=False):
    from contextlib import nullcontext

    import concourse.tile as tile
    from concourse import bacc, mybir
    from concourse._compat import get_trn_type
    from concourse.hw_specs import get_activation_tables

    st = STYLES[style] if isinstance(style, str) else style

    f32 = mybir.dt.float32
    f32r = mybir.dt.float32r
    bf16 = mybir.dt.bfloat16
    AF = mybir.ActivationFunctionType
    OP = mybir.AluOpType

    nc = bacc.Bacc(get_trn_type() or "TRN2", target_bir_lowering=False, debug=debug)

    # activation-function set with exp+ln co-resident (also copy/square/
    # identity): one explicit load, no per-iteration table thrash
    tables = list(get_activation_tables(nc.m.arch).items())
    actset = next(
        i for i, (_, fs) in enumerate(tables) if AF.Exp in fs and AF.Ln in fs
    )

    x_d = nc.dram_tensor("x", [PD, NCH * R], bf16, kind="ExternalInput")
    w_d = nc.dram_tensor("w", [PD, WAUX], bf16, kind="ExternalInput")
    sel_d = nc.dram_tensor("sel", [E, E * P], f32r, kind="ExternalInput")
    out_d = nc.dram_tensor("out", [P, R], bf16, kind="ExternalOutput")

    B0 = -math.log1p(EPS)  # folds the 1/(rev_w+eps)=1/(1+eps) denorm scale

    def eng(name):
        return {"vector": nc.vector, "gpsimd": nc.gpsimd, "act": nc.scalar}[name]

    with tile.TileContext(nc) as tc:
        with (
            tc.tile_pool(name="const", bufs=1) as cp,
            tc.tile_pool(name="big", bufs=2) as bp,
            tc.tile_pool(name="sm", bufs=2) as sp,
            tc.tile_pool(name="psall", bufs=1, space="PSUM") as ps_p,
        ):
            # --- one-time constants + act table load (outside the timing loop) ---
            onesR = cp.tile([1, R], f32r, tag="onesR")      # all-ones lhsT/rhs row
            onesRb = cp.tile([1, R], bf16, tag="onesRb")    # bf16 rhs for bias folds
            onesRf = cp.tile([1, R], f32, tag="onesRf")
            invL = cp.tile([PD, 1], bf16, tag="invL")       # stats lhsT col
            invLn = cp.tile([PD, 1], bf16, tag="invLn")     # negated (for -mean)
            invLf = cp.tile([PD, 1], f32, tag="invLf")
            ones4 = cp.tile([E, 1], f32r, tag="ones4")      # score-sum lhsT
            ones4f = cp.tile([E, 1], f32, tag="ones4f")
            cbias = cp.tile([1, 2], f32, tag="cbias")
            zbias = cp.tile([PD, 1], f32, tag="zbias")
            nc.gpsimd.memset(onesRf, 1.0)
            nc.vector.tensor_copy(onesR, onesRf)
            nc.vector.tensor_copy(onesRb, onesRf)
            nc.gpsimd.memset(invLf, 1.0 / L)
            nc.vector.tensor_copy(invL, invLf)
            invLf2 = cp.tile([PD, 1], f32, tag="invLf2")
            nc.gpsimd.memset(invLf2, -1.0 / L)
            nc.vector.tensor_copy(invLn, invLf2)
            nc.gpsimd.memset(ones4f, 1.0)
            nc.vector.tensor_copy(ones4, ones4f)
            nc.gpsimd.memset(cbias[:, 0:1], EPS)
            nc.gpsimd.memset(cbias[:, 1:2], B0)
            nc.vector.memset(zbias, 0.0)
            ones_r = onesR[:, 0:PD]
            sel = cp.tile([E, E * P], f32r, tag="sel")      # one-hot row selectors
            nc.scalar.dma_start(out=sel, in_=sel_d[:])
            ld = mybir.InstLoadActFuncSet(
                name=nc.get_next_instruction_name(), ins=[], outs=[]
            )
            ld.act_func_set_id = actset
            ld.engine = nc.scalar.engine
            nc.scalar.add_instruction(ld)

            with tc.For_i(0, loop_n, 1, staggered_reset=STAGGER) if loop_n > 1 else nullcontext():
                # --- DMAs: x on sync ring; aux row + weights on scalar ring ---
                xs = bp.tile([PD, NCH * R], bf16, tag="xs")
                wsb = cp.tile([PD, WAUX], bf16, tag="wsb")
                H = NCH * R // 2
                nc.sync.dma_start(out=xs[:, 0:H], in_=x_d[:, 0:H])
                nc.sync.dma_start(out=xs[:, H:], in_=x_d[:, H:])
                nc.sync.dma_start(out=wsb, in_=w_d[:])
                aux = wsb[0:1, WTOT : WTOT + E * P + E]

                c10 = wsb[:, 0 * WCOLS : 1 * WCOLS]
                c20 = wsb[:, 1 * WCOLS : 2 * WCOLS]
                c11 = wsb[:, 2 * WCOLS : 3 * WCOLS]
                c21 = wsb[:, 3 * WCOLS : 4 * WCOLS]
                ww0 = wsb[:, 4 * WCOLS : 5 * WCOLS]
                ww1 = wsb[:, 5 * WCOLS : 6 * WCOLS]
                wg = wsb[:, 6 * WCOLS : 6 * WCOLS + NCH * E]
                gbrow = aux[:, E * P : E * P + E]

                # --- RevIN stats: -mean | E[x^2] in ONE interleaved PSUM group ---
                sq = bp.tile([PD, NCH * R], bf16, tag="sq")
                bankS = ps_p.tile([1, R], f32, tag="bankS")
                bankS2 = ps_p.tile([1, R], f32, tag="bankS2")
                negm_ps = bankS[:, 0:R]
                ex2_ps = bankS2[:, 0:R]
                # x-sum matmuls first (their group closes early and unblocks
                # the whole istd chain); squares + their matmuls trail
                for c in range(NCH):
                    cs = slice(c * R, (c + 1) * R)
                    nc.vector.tensor_mul(sq[:, cs], xs[:, cs], xs[:, cs])
                    nc.tensor.matmul(
                        negm_ps, invLn, xs[:, cs], start=(c == 0), stop=(c == NCH - 1)
                    )
                for c in range(NCH):
                    cs = slice(c * R, (c + 1) * R)
                    nc.tensor.matmul(
                        ex2_ps, invL, sq[:, cs], start=(c == 0), stop=(c == NCH - 1)
                    )

                # --- istd/stdev chain (exp/ln in place of sqrt) ---
                tsm = sp.tile([1, R], f32, tag="tsm")
                varsm = sp.tile([1, R], f32, tag="varsm")
                lvsm = sp.tile([1, R], f32, tag="lvsm")
                sdrsm = sp.tile([1, R], f32, tag="sdrsm")
                acr = sp.tile([1, 2 * R], f32r, tag="acr")    # [istd | -mean]
                dmrow = sp.tile([1, R], f32r, tag="dmrow")      # dar row
                pac = ps_p.tile([PD, 2 * R], f32, tag="pac")  # [A bcast | CM bcast]
                ACsb = bp.tile([PD, 2 * R], bf16, tag="ACsb")
                # CM = bcast(-mean) is ready long before istd: the centering
                # adds run during the ln/exp chain. negm copy on DVE so ACT
                # starts the mean^2 concurrently; ACT queue is chain-first
                # (t, lv, istd) with the off-chain copies behind.
                nc.vector.tensor_copy(acr[:, R:], negm_ps)
                nc.tensor.matmul(pac[:, R:], ones_r, acr[:, R:], start=True, stop=True)
                nc.scalar.activation(ACsb[:, R:], pac[:, R:], AF.Copy)
                nc.vector.tensor_mul(tsm, acr[:, R:], acr[:, R:])
                nc.vector.scalar_tensor_tensor(
                    varsm, tsm, -1.0, ex2_ps, op0=OP.mult, op1=OP.add
                )
                nc.scalar.activation(lvsm, varsm, AF.Ln, bias=cbias[:, 0:1])
                nc.scalar.activation(acr[:, 0:R], lvsm, AF.Exp, scale=-0.5)
                nc.tensor.matmul(pac[:, 0:R], ones_r, acr[:, 0:R], start=True, stop=True)
                nc.scalar.activation(ACsb[:, 0:R], pac[:, 0:R], AF.Copy)
                nc.vector.scalar_tensor_tensor(
                    sdrsm, varsm, EPS, acr[:, 0:R], op0=OP.add, op1=OP.mult
                )

                # --- tiles ---
                xsc = bp.tile([PD, NCH * R], bf16, tag="xsc")
                xn = bp.tile([PD, NCH * R], bf16, tag="xn")
                x2 = bp.tile([PD, NCH * R], bf16, tag="x2")
                eT = bp.tile([PD, NCH * R], bf16, tag="eT")
                psib = bp.tile([PD, NCH * R], bf16, tag="psib")
                ptP = ps_p.tile([P, 2 * R], f32, tag="ptP")   # pt0|pt1 one group
                pwP = ps_p.tile([P, 2 * R], f32, tag="pwP")   # pw0|pw1 one group
                pt0 = ptP[:, 0:R]
                pt1 = ptP[:, R:]
                pw0 = pwP[:, 0:R]
                pw1 = pwP[:, R:]
                pgt = ps_p.tile([E, 2 * R], f32, tag="pgt")
                pg = pgt[:, 0:R]
                prs = pgt[0:1, R : R + R]
                pG01 = ps_p.tile([P, 2 * R], f32, tag="pG01")
                pG23 = ps_p.tile([P, 2 * R], f32, tag="pG23")
                pGs = [pG01[:, 0:R], pG01[:, R:], pG23[:, 0:R], pG23[:, R:]]
                # time-disjoint alias: DA broadcast reuses the A-bcast region
                # (its readers all finish before the score chain)
                pdadc = pac[0:P, 0:R]
                expg = sp.tile([E, R], f32r, tag="expg")
                rinvsm = sp.tile([1, R], f32, tag="rinvsm")
                G01sb = bp.tile([P, 2 * R], bf16, tag="G01sb")
                G23sb = bp.tile([P, 2 * R], bf16, tag="G23sb")
                DAsb = bp.tile([P, R], bf16, tag="DAsb")

                bmm = [aux[:, e * P : (e + 1) * P] for e in range(E)]
                # centering (xs - mean) for all chunks: overlaps the istd chain
                for c in range(NCH):
                    cs = slice(c * R, (c + 1) * R)
                    CM_ = pac[:, R:] if c == 0 else ACsb[:, R:]
                    xe = nc.vector if c == 0 else nc.gpsimd
                    xe.tensor_add(xsc[:, cs], xs[:, cs], CM_)

                # pass A: xn/x2/eT + gate & taylor matmuls. Wave matmuls are
                # deferred so PE reaches the gate close + score broadcasts
                # right after xn3 instead of behind the psi backlog.
                for c in range(NCH):
                    cs = slice(c * R, (c + 1) * R)
                    cp_ = slice(c * P, (c + 1) * P)
                    A_ = pac[:, 0:R] if c == 0 else ACsb[:, 0:R]
                    last = c == NCH - 1
                    nc.vector.tensor_mul(xn[:, cs], xsc[:, cs], A_)
                    if c == 0:
                        nc.tensor.matmul(pg, gbrow, onesRb, start=True, stop=False)
                    nc.tensor.matmul(
                        pg, wg[:, c * E : (c + 1) * E], xn[:, cs],
                        start=False, stop=last,
                    )
                    if last:
                        # gate scores on ACT before eT3: the score chain and
                        # G broadcasts overlap the wave tail
                        nc.scalar.activation(expg, pg, AF.Exp, bias=zbias[0:E])
                    nc.gpsimd.tensor_mul(x2[:, cs], xn[:, cs], xn[:, cs])
                    nc.scalar.activation(eT[:, cs], x2[:, cs], AF.Exp,
                                         bias=zbias, scale=-0.5)
                    nc.tensor.matmul(pt0, c10[:, cp_], xn[:, cs],
                                     start=(c == 0), stop=False)
                    nc.tensor.matmul(pt1, c11[:, cp_], xn[:, cs],
                                     start=False, stop=False)
                    nc.tensor.matmul(pt0, c20[:, cp_], x2[:, cs],
                                     start=False, stop=False)
                    nc.tensor.matmul(pt1, c21[:, cp_], x2[:, cs],
                                     start=False, stop=False)
                    if last:
                        nc.tensor.matmul(pt0, bmm[0], onesRb, start=False, stop=False)
                        nc.tensor.matmul(pt1, bmm[1], onesRb, start=False, stop=True)
                        nc.tensor.matmul(prs, ones4, expg, start=True, stop=True)
                        for e in range(E):
                            nc.tensor.matmul(pGs[e], sel[:, e * P : (e + 1) * P],
                                             expg, start=True, stop=True)

                # pass B: psi + wave matmuls (terminal chain) + score scale
                nc.vector.reciprocal(rinvsm, prs)
                nc.vector.tensor_mul(dmrow, sdrsm, rinvsm)
                for c in range(NCH):
                    cs = slice(c * R, (c + 1) * R)
                    cp_ = slice(c * P, (c + 1) * P)
                    last = c == NCH - 1
                    nc.vector.scalar_tensor_tensor(
                        psib[:, cs], x2[:, cs], -1.0, eT[:, cs],
                        op0=OP.add, op1=OP.mult,
                    )
                    nc.tensor.matmul(pw0, ww0[:, cp_], psib[:, cs],
                                     start=(c == 0), stop=False)
                    nc.tensor.matmul(pw1, ww1[:, cp_], psib[:, cs],
                                     start=False, stop=False)
                    if c == 1:
                        nc.tensor.matmul(pdadc, ones_r[:, 0:P], dmrow,
                                         start=True, stop=True)
                    if last:
                        nc.tensor.matmul(pw0, bmm[2], onesRb, start=False, stop=False)
                        nc.tensor.matmul(pw1, bmm[3], onesRb, start=False, stop=True)

                # SBUF copies of the broadcasts (single-PSUM-operand rule)
                nc.scalar.activation(G01sb, pG01, AF.Copy)
                nc.scalar.activation(G23sb, pG23, AF.Copy)
                nc.scalar.activation(DAsb, pdadc, AF.Copy)

                # --- mixture + denorm, feature-major ---
                # taylor-side pre = s1*DA + mean via DVE+Pool (off the
                # terminal path); wave side m23 -> s2 -> s2*DA -> +pre
                m01 = bp.tile([P, 2 * R], bf16, tag="m01")
                m23 = bp.tile([P, 2 * R], bf16, tag="m23")
                s1 = bp.tile([P, R], bf16, tag="s1")
                s2 = bp.tile([P, R], bf16, tag="s2")
                pre = bp.tile([P, R], bf16, tag="pre")
                outp = bp.tile([P, R], bf16, tag="outp")
                nc.vector.tensor_mul(m01, ptP, G01sb)
                nc.vector.tensor_add(s1, m01[:, 0:R], m01[:, R:])
                nc.gpsimd.tensor_mul(s1, s1, DAsb)
                nc.gpsimd.tensor_sub(pre, s1, ACsb[0:P, R:])
                nc.vector.tensor_mul(m23, pwP, G23sb)
                nc.vector.tensor_add(s2, m23[:, 0:R], m23[:, R:])
                nc.vector.tensor_mul(s2, s2, DAsb)
                nc.vector.tensor_add(outp, s2, pre)
                nc.sync.dma_start(out=out_d[:], in_=outp)

    nc.compile()
    return nc


def _chunked(wT):
    """[L, M] -> [128, NCH*M], column block c holds rows l = c*128..(c+1)*128."""
    Lx, M = wT.shape
    return np.ascontiguousarray(
        wT.reshape(NCH, PD, M).transpose(1, 0, 2).reshape(PD, NCH * M)
    )


def _host_prep(inputs):
    import ml_dtypes

    f = np.float32
    bf = ml_dtypes.bfloat16
    g = {k: np.asarray(v, f) for k, v in inputs.items()}

    bn_scale = MH / math.sqrt(1.0 + BN_EPS)
    wparts = [
        _chunked(np.ascontiguousarray(g["t0_coeffs"][:, :, 1].T)),
        _chunked(np.ascontiguousarray(g["t0_coeffs"][:, :, 2].T)),
        _chunked(np.ascontiguousarray(g["t1_coeffs"][:, :, 1].T)),
        _chunked(np.ascontiguousarray(g["t1_coeffs"][:, :, 2].T)),
        _chunked(
            np.ascontiguousarray((g["w0_ww"] * g["w0_gamma"][:, None] * bn_scale).T)
        ),
        _chunked(
            np.ascontiguousarray((g["w1_ww"] * g["w1_gamma"][:, None] * bn_scale).T)
        ),
        _chunked(np.ascontiguousarray(g["gate_w"].T)),
    ]
    w_h = np.concatenate(wparts, axis=1).astype(bf)
    assert w_h.shape == (PD, WTOT)

    aux_h = np.zeros((1, E * P + E), f)
    aux_h[0, 0:P] = (
        g["t0_coeffs"][:, :, 0].sum(axis=1, dtype=np.float64) + g["t0_bias"][0]
    ).astype(f)
    aux_h[0, P : 2 * P] = (
        g["t1_coeffs"][:, :, 0].sum(axis=1, dtype=np.float64) + g["t1_bias"][0]
    ).astype(f)
    aux_h[0, 2 * P : 3 * P] = g["w0_beta"]
    aux_h[0, 3 * P : 4 * P] = g["w1_beta"]
    aux_h[0, 4 * P : 4 * P + E] = g["gate_b"] + np.float32(math.log1p(EPS))
    wa_h = np.zeros((PD, E * P + E), f)
    wa_h[0, :] = aux_h[0]
    w_h = np.concatenate([w_h, wa_h.astype(bf)], axis=1)
    assert w_h.shape == (PD, WAUX)

    sel_h = np.zeros((E, E * P), f)
    for e in range(E):
        sel_h[e, e * P : (e + 1) * P] = 1.0
    common = {"w": w_h, "sel": sel_h}

    x = g["x"]
    xcores = []
    for i in range(NCORES):
        xc = x[i * BPC : (i + 1) * BPC]  # [BPC, L, N]
        xcores.append(
            np.ascontiguousarray(
                xc.reshape(BPC, NCH, PD, N).transpose(2, 1, 0, 3).reshape(PD, NCH * R)
            ).astype(bf)
        )
    return common, xcores


def _fast_ok(inputs):
    try:
        return (
            np.all(np.asarray(inputs["w0_scale"]) == 1.0)
            and np.all(np.asarray(inputs["w1_scale"]) == 1.0)
            and np.all(np.asarray(inputs["w0_trans"]) == 0.0)
            and np.all(np.asarray(inputs["w1_trans"]) == 0.0)
            and np.all(np.asarray(inputs["rev_w"]) == 1.0)
            and np.all(np.asarray(inputs["rev_b"]) == 0.0)
        )
    except Exception:
        return False


def _numpy_ref(inputs):
    """Exact general fallback (host numpy), mirrors the reference module."""
    g = {k: np.asarray(v, np.float32) for k, v in inputs.items()}
    x = g["x"]
    mean = x.mean(axis=1, keepdims=True)
    stdev = np.sqrt(x.var(axis=1, keepdims=True) + np.float32(EPS))
    xn = (x - mean) / stdev * g["rev_w"] + g["rev_b"]
    xf = xn.transpose(0, 2, 1).reshape(B * N, L)
    logits = xf @ g["gate_w"].T + g["gate_b"]
    logits -= logits.max(axis=-1, keepdims=True)
    elg = np.exp(logits)
    score = elg / elg.sum(axis=-1, keepdims=True)

    def taylor(c, b):
        y = np.full((B * N, P), c[:, :, 0].sum(axis=1), np.float32)
        y += xf @ c[:, :, 1].T + (xf * xf) @ c[:, :, 2].T
        return y + b

    def wave(s, t, w, gam, bet):
        y = np.empty((B * N, P), np.float32)
        for i0 in range(0, B * N, 128):
            xs = (xf[i0 : i0 + 128, None, :] - t[None]) / s[None]
            x2 = xs * xs
            psi = np.float32(MH) * (x2 - 1.0) * np.exp(-0.5 * x2)
            y[i0 : i0 + 128] = np.einsum("bpl,pl->bp", psi, w)
        return (y / np.sqrt(np.float32(1.0 + BN_EPS))) * gam + bet

    eo = np.stack(
        [
            taylor(g["t0_coeffs"], g["t0_bias"][0]),
            taylor(g["t1_coeffs"], g["t1_bias"][0]),
            wave(g["w0_scale"], g["w0_trans"], g["w0_ww"], g["w0_gamma"], g["w0_beta"]),
            wave(g["w1_scale"], g["w1_trans"], g["w1_ww"], g["w1_gamma"], g["w1_beta"]),
        ],
        axis=-1,
    )
    pred = np.einsum("bpE,bE->bp", eo, score)
    pred = pred.reshape(B, N, P).transpose(0, 2, 1)
    out = ((pred - g["rev_b"]) / (g["rev_w"] + np.float32(EPS))) * stdev + mean
    return out.astype(np.float32)


def run(inputs, trace=False):
    """Run the Bass kernel on 8 cores. Returns (out [B,P,N], exec_time_ns|None)."""
    from concourse.bass_utils import run_bass_kernel_spmd

    if "nc" not in _NC_CACHE:
        _NC_CACHE["nc"] = _build_nc()
    nc = _NC_CACHE["nc"]
    common, xcores = _host_prep(inputs)
    in_maps = [dict(common, x=xcores[i]) for i in range(NCORES)]
    try:
        res = run_bass_kernel_spmd(nc, in_maps, list(range(NCORES)), trace=trace)
    except ModuleNotFoundError:
        res = run_bass_kernel_spmd(nc, in_maps, list(range(NCORES)), trace=False)
    out = np.empty((B, P, N), np.float32)
    for i in range(NCORES):
        o = np.asarray(res.results[i]["out"]).astype(np.float32).reshape(P, BPC, N)
        out[i * BPC : (i + 1) * BPC] = o.transpose(1, 0, 2)
    return out, res.exec_time_ns


def kernel(**inputs):
    if not _fast_ok(inputs):
        return _numpy_ref(inputs)
    out, _ = run(inputs)
    return out
